# revision 1
# baseline (speedup 1.0000x reference)
"""Multi-head causal self-attention with RoPE on 8 Trainium2 NeuronCores.

Problem: x[2,2048,2048], wq/wk/wv/wo[2048,2048] fp32, 16 heads (hd=128),
interleaved RoPE, causal softmax, Megatron-style tensor parallelism over
heads: 2 heads per core, wo row-sharded, partial outputs summed on host.

All matmuls run as float32r (fp32 rounded to 11-bit mantissa; ~1 cycle/row
warm when back-to-back). Host pre-rounds DRAM inputs to f32r; on-device
producers write f32r directly.

Layout strategy (per core, per batch b):
  - host supplies xT = x^T [d, s] (f32r) and weight slices pre-transposed
  - projections: qT,kT per head via lhsT=w-tile [d,e], rhs=xT [d,s512]
    -> q^T,k^T [e=128, s] directly; v natural [s, e] via lhsT=xT-subtile;
    RoPE fused right after each projection chunk:
    qrotT = RotL.T @ qT (signed pair-swap as a matmul), then
    q_roped = qT*cosT + qrotT*sinT on DVE (tables indexed [e, s])
  - attention per (b, j-block of 512 q), heads interleaved:
      scoresT[kv=128, q=512] = kT-tile.T @ qT-block   (one matmul, d=128)
      staircase tiles compute only valid columns [delta:512]
      attn = exp(scoresT) on ACT (-> f32r); triangle mask on 128-col band
      oT[d, q] += v-tile.T @ attn ; rowsumB[128, q] += ones.T @ attn
      oT_norm = oT * reciprocal_approx_fast(rowsumB)  (-> f32r)
  - output projection: yT[e, s] = sum_ct woT-tile.T @ oT  -> DRAM
  - host: y = sum over cores of yT^T
"""

import os
import sys

for _p in ("/opt/trn_rl_repo", "/root/.axon_site/_ro/trn_rl_repo"):
    if os.path.isdir(_p) and _p not in sys.path:
        sys.path.append(_p)

import numpy as np

import concourse.bacc as bacc
import concourse.mybir as mybir
import concourse.tile as tile
from concourse.alu_op_type import AluOpType
from concourse.bass_utils import run_bass_kernel_spmd

F32 = mybir.dt.float32
F32R = mybir.dt.float32r
BF16 = mybir.dt.bfloat16

B, S, D = 2, 2048, 2048
H, HD = 16, 128
NCORES = 8
HPC = H // NCORES            # heads per core = 2
CPC = HPC * HD               # channels per core = 256
P = 128
SC = 512                     # s-chunk for projections / q-block for attention
NSC = S // SC                # 4
NDT = D // P                 # 16 contraction tiles
NG = 2                       # x-tile DMA group: d-tiles per DMA
ROPE_THETA = 10000.0

Exp = mybir.ActivationFunctionType.Exp

last_exec_time_ns = None
_nc_cache = None


def _round_f32r(x):
    u = np.ascontiguousarray(x, dtype=np.float32).view(np.uint32)
    r = (u + np.uint32(0x7FF) + ((u >> np.uint32(12)) & np.uint32(1))) \
        & np.uint32(0xFFFFF000)
    return r.view(np.float32)


def _build_nc():
    nc = bacc.Bacc("TRN2", target_bir_lowering=False, debug=False)

    xT = nc.dram_tensor("xT", [B, D, S], F32R, kind="ExternalInput")
    wqkvT = nc.dram_tensor("wqkvT", [D, 6 * P], F32R, kind="ExternalInput")
    woT = nc.dram_tensor("woT", [CPC, D], F32R, kind="ExternalInput")
    cosT = nc.dram_tensor("cosT", [HD, S], F32, kind="ExternalInput")
    sinT = nc.dram_tensor("sinT", [HD, S], F32, kind="ExternalInput")
    rotL = nc.dram_tensor("rotL", [HD, HD], F32R, kind="ExternalInput")
    trimask = nc.dram_tensor("trimask", [P, P], BF16, kind="ExternalInput")
    ones = nc.dram_tensor("ones", [P, P], F32R, kind="ExternalInput")
    yT = nc.dram_tensor("yT", [B, D, S], F32, kind="ExternalOutput")

    xTr = xT.rearrange("b (o p) s -> b p o s", p=P)

    with tile.TileContext(nc) as tc:
        with tc.tile_pool(name="const", bufs=1) as constp, \
             tc.tile_pool(name="xp", bufs=9) as xp, \
             tc.tile_pool(name="qk", bufs=1) as qkp, \
             tc.tile_pool(name="vp", bufs=1) as vp, \
             tc.tile_pool(name="op", bufs=1) as op_, \
             tc.tile_pool(name="attn", bufs=4) as attnp, \
             tc.tile_pool(name="tmp", bufs=2) as tmpp, \
             tc.tile_pool(name="yt", bufs=2) as ytp, \
             tc.tile_pool(name="ps", bufs=4, space="PSUM") as psp, \
             tc.tile_pool(name="acc", bufs=4, space="PSUM") as accp:

            # ---- constants (wq split per d-tile so matmuls start early;
            #      the rest deferred until after the first x-chunk DMAs) ----
            wq_sb = constp.tile([P, NDT, 6 * P], F32R)
            wqr = wqkvT.rearrange("(o p) e -> p o e", p=P)
            for dt in range(NDT):
                nc.sync.dma_start(wq_sb[:, dt, :], wqr[:, dt, :])
            wo_sb = constp.tile([P, CPC // P, D], F32R)
            cos_sb = constp.tile([P, S], F32)
            sin_sb = constp.tile([P, S], F32)
            rot_sb = constp.tile([P, P], F32R)
            mask_sb = constp.tile([P, P], BF16)
            ones_sb = constp.tile([P, P], F32R)

            def load_rest_of_consts():
                nc.sync.dma_start(rot_sb[:], rotL[:])
                nc.sync.dma_start(cos_sb[:], cosT[:])
                nc.sync.dma_start(sin_sb[:], sinT[:])
                nc.sync.dma_start(mask_sb[:], trimask[:])
                nc.sync.dma_start(ones_sb[:], ones[:])
                nc.sync.dma_start(wo_sb[:], woT.rearrange("(o p) e -> p o e", p=P))

            for b in range(B):
                # ---- projections (+ fused RoPE) ----
                # qkT[e] for e in {q_h0, q_h1, k_h0, k_h1}: [128, S] transposed
                qkT = [qkp.tile([P, S], F32R, tag=f"qk{e}", name=f"qkT{e}")
                       for e in range(4)]
                # v natural [s_in=128, s_out=16, ch=256]
                v_sb = vp.tile([P, NDT, CPC], F32R, tag="v")
                for sc in range(NSC):
                    xts = []
                    for g in range(NDT // NG):
                        xt = xp.tile([P, NG, SC], F32R, tag="xt")
                        nc.gpsimd.dma_start(
                            xt[:], xTr[b, :, g * NG:(g + 1) * NG,
                                       sc * SC:(sc + 1) * SC])
                        xts.append(xt)
                    if b == 0 and sc == 0:
                        load_rest_of_consts()
                    for e in range(4):
                        pq = accp.tile([P, SC], F32, tag="acc")
                        for dt in range(NDT):
                            nc.tensor.matmul(pq[:],
                                             wq_sb[:, dt, e * P:(e + 1) * P],
                                             xts[dt // NG][:, dt % NG, :],
                                             start=(dt == 0), stop=(dt == NDT - 1))
                        sl = slice(sc * SC, (sc + 1) * SC)
                        nc.scalar.copy(qkT[e][:, sl], pq[:])
                        # RoPE for this chunk, overlapped with projections
                        pr = psp.tile([P, SC], F32, tag="ps")
                        nc.tensor.matmul(pr[:], rot_sb[:], qkT[e][:, sl],
                                         start=True, stop=True)
                        tmp = tmpp.tile([P, SC], F32, tag="ropetmp")
                        nc.vector.tensor_tensor(tmp[:], pr[:], sin_sb[:, sl],
                                                AluOpType.mult)
                        nc.vector.tensor_tensor(qkT[e][:, sl], qkT[e][:, sl],
                                                cos_sb[:, sl], AluOpType.mult)
                        nc.vector.tensor_tensor(qkT[e][:, sl], qkT[e][:, sl],
                                                tmp[:], AluOpType.add)
                    for ss in range(SC // P):
                        pv = accp.tile([P, SC], F32, tag="acc")
                        pvv = pv[:, :CPC]
                        for dt in range(NDT):
                            nc.tensor.matmul(pvv,
                                             xts[dt // NG][:, dt % NG,
                                                           ss * P:(ss + 1) * P],
                                             wq_sb[:, dt, 4 * P:6 * P],
                                             start=(dt == 0), stop=(dt == NDT - 1))
                        nc.scalar.copy(v_sb[:, sc * (SC // P) + ss, :], pvv)

                # ---- attention: j outer, heads interleaved ----
                oT = op_.tile([P, HPC, S], F32R, tag="o")
                for j in range(NSC):
                    jsl = slice(j * SC, (j + 1) * SC)
                    n_kv = (SC // P) * (j + 1)
                    for h in range(HPC):
                        qTh, kTh = qkT[h], qkT[2 + h]
                        po = accp.tile([P, SC], F32, tag="acc")
                        prs = accp.tile([P, SC], F32, tag="acc")
                        for t in range(n_kv):
                            dp = t - (SC // P) * j
                            dlt = max(dp, 0) * P  # first valid column
                            vsl = slice(j * SC + dlt, (j + 1) * SC)
                            pscore = psp.tile([P, SC], F32, tag="ps")
                            nc.tensor.matmul(pscore[:, dlt:],
                                             kTh[:, t * P:(t + 1) * P],
                                             qTh[:, vsl],
                                             start=True, stop=True)
                            attn = attnp.tile([P, SC], F32R, tag="attn")
                            nc.scalar.activation(attn[:, dlt:], pscore[:, dlt:],
                                                 Exp, bias=0.0, scale=1.0)
                            if dp >= 0:  # triangle mask on the 128-col band
                                nc.vector.tensor_tensor(
                                    attn[:, dlt:dlt + P], attn[:, dlt:dlt + P],
                                    mask_sb[:], AluOpType.mult)
                            nc.tensor.matmul(po[:, dlt:],
                                             v_sb[:, t, h * HD:(h + 1) * HD],
                                             attn[:, dlt:],
                                             start=(t == 0), stop=(t == n_kv - 1),
                                             skip_group_check=True)
                            nc.tensor.matmul(prs[:, dlt:], ones_sb[:],
                                             attn[:, dlt:],
                                             start=(t == 0), stop=(t == n_kv - 1),
                                             skip_group_check=True)
                        recip = tmpp.tile([P, SC], F32, tag="recip")
                        nc.vector.reciprocal_approx_fast(recip[:], prs[:])
                        nc.vector.tensor_tensor(oT[:, h, jsl], po[:], recip[:],
                                                AluOpType.mult)

                    # ---- output projection for this q-block:
                    #      yT[e, jsl] = sum_ct woT.T @ oT ----
                    for eh in range(NDT // 2):
                        yt = ytp.tile([P, 2, SC], F32, tag="yt")
                        for si in range(2):
                            et = eh * 2 + si
                            py = accp.tile([P, SC], F32, tag="acc")
                            for ct in range(HPC):
                                nc.tensor.matmul(
                                    py[:],
                                    wo_sb[:, ct, et * P:(et + 1) * P],
                                    oT[:, ct, jsl],
                                    start=(ct == 0), stop=(ct == HPC - 1))
                            nc.scalar.copy(yt[:, si, :], py[:])
                        nc.sync.dma_start(
                            yT[b, eh * 2 * P:(eh + 1) * 2 * P, jsl]
                            .rearrange("(n p) q -> p n q", p=P),
                            yt[:])
    nc.finalize()
    return nc


def _host_inputs(x, wq, wk, wv, wo):
    """Build per-core input maps (host-side shard + transform)."""
    scale = 1.0 / np.sqrt(np.float32(HD))

    xTr = _round_f32r(np.ascontiguousarray(x.transpose(0, 2, 1)))

    # RoPE tables in [e, s] layout (same for every head)
    inv_freq = 1.0 / (ROPE_THETA ** (np.arange(0, HD, 2, dtype=np.float64) / HD))
    ang = np.arange(S, dtype=np.float64)[None, :] * inv_freq[:, None]  # [64, S]
    cosT = np.repeat(np.cos(ang), 2, axis=0).astype(np.float32)  # [128, S]
    sinT = np.repeat(np.sin(ang), 2, axis=0).astype(np.float32)

    # signed pair-swap: qrot[2i] = -q[2i+1], qrot[2i+1] = q[2i]
    # matmul computes qrot[m, s] = sum_k rotL[k, m] q[k, s]
    rotL = np.zeros((HD, HD), dtype=np.float32)
    for i in range(HD // 2):
        rotL[2 * i + 1, 2 * i] = -1.0
        rotL[2 * i, 2 * i + 1] = 1.0

    import ml_dtypes
    r = np.arange(P)[:, None]
    c = np.arange(P)[None, :]
    trimask = (c >= r).astype(ml_dtypes.bfloat16)  # [128,128] upper-right valid

    wq_s = _round_f32r(wq * scale)
    wk_s = _round_f32r(wk)
    wv_s = _round_f32r(wv)
    wo_s = _round_f32r(wo)

    in_maps = []
    for cix in range(NCORES):
        rows = slice(cix * CPC, (cix + 1) * CPC)  # head-channel rows
        blocks = []
        for h in range(HPC):
            hr = slice((cix * HPC + h) * HD, (cix * HPC + h + 1) * HD)
            blocks.append(wq_s[hr])   # q_h: [128, D]
        for h in range(HPC):
            hr = slice((cix * HPC + h) * HD, (cix * HPC + h + 1) * HD)
            blocks.append(wk_s[hr])
        blocks.append(wv_s[rows])     # v both heads: [256, D]
        wqkvT = np.ascontiguousarray(
            np.concatenate(blocks, axis=0).T)  # [D, 768]
        woT = np.ascontiguousarray(wo_s[:, rows].T)  # [256, D]
        in_maps.append({
            "xT": xTr,
            "wqkvT": wqkvT,
            "woT": woT,
            "cosT": cosT,
            "sinT": sinT,
            "rotL": rotL,
            "trimask": trimask,
            "ones": np.ones((P, P), dtype=np.float32),
        })
    return in_maps


def _get_nc():
    global _nc_cache
    if _nc_cache is None:
        _nc_cache = _build_nc()
    return _nc_cache


def kernel(x, wq, wk, wv, wo, _trace=False):
    global last_exec_time_ns
    nc = _get_nc()
    in_maps = _host_inputs(np.asarray(x, dtype=np.float32),
                           np.asarray(wq, dtype=np.float32),
                           np.asarray(wk, dtype=np.float32),
                           np.asarray(wv, dtype=np.float32),
                           np.asarray(wo, dtype=np.float32))
    res = run_bass_kernel_spmd(nc, in_maps, core_ids=list(range(NCORES)),
                               trace=_trace)
    last_exec_time_ns = res.exec_time_ns
    y = np.zeros((B, S, D), dtype=np.float64)
    for cix in range(NCORES):
        y += res.results[cix]["yT"].transpose(0, 2, 1).astype(np.float64)
    return y.astype(np.float32)



# revision 3
# speedup vs baseline: 1.2478x; 1.2478x over previous
"""Multi-head causal self-attention with RoPE on 8 Trainium2 NeuronCores.

Problem: x[2,2048,2048], wq/wk/wv/wo[2048,2048] fp32, 16 heads (hd=128),
interleaved RoPE, causal softmax, Megatron-style tensor parallelism over
heads: 2 heads per core, wo row-sharded, partial outputs summed on host.

v2: all-bf16 matmul datapath (tolerance 2e-2 >> bf16 error), restructured
for weight-load amortization and near-100% PE occupancy:
  - host converts x/weights to bf16; xT resident in SBUF per batch
  - projections loop (e-tile, dt): each weight tile streams 2x512 cols
    per load; q/k produced transposed [e,s] with fused RoPE; v natural
  - attention per (b, j-block, head): scoresT = kT.T @ qT staircase,
    exp on ACT -> bf16 attn tiles, AV accumulation; rowsum matmuls
    deferred per (j,h) so the `ones` weights stay stationary
  - output projection after attention with wo tiles stationary across
    4 s-chunks; y partial sums written bf16, summed on host
"""

import os
import sys

for _p in ("/opt/trn_rl_repo", "/root/.axon_site/_ro/trn_rl_repo"):
    if os.path.isdir(_p) and _p not in sys.path:
        sys.path.append(_p)

import numpy as np

import concourse.bacc as bacc
import concourse.mybir as mybir
import concourse.tile as tile
from concourse.alu_op_type import AluOpType
from concourse.bass_utils import run_bass_kernel_spmd

F32 = mybir.dt.float32
BF16 = mybir.dt.bfloat16

B, S, D = 2, 2048, 2048
H, HD = 16, 128
NCORES = 8
HPC = H // NCORES            # heads per core = 2
CPC = HPC * HD               # channels per core = 256
P = 128
SC = 512                     # q-block for attention / PSUM bank width
NSC = S // SC                # 4
W = 1024                     # projection s-chunk width
NW = S // W                  # 2 chunks per batch
NDT = D // P                 # 16 contraction tiles
XG = 4                       # d-tiles per x DMA tile
ROPE_THETA = 10000.0

Exp = mybir.ActivationFunctionType.Exp

last_exec_time_ns = None
_nc_cache = None


def _build_nc():
    nc = bacc.Bacc("TRN2", target_bir_lowering=False, debug=False)

    xT = nc.dram_tensor("xT", [B, D, S], BF16, kind="ExternalInput")
    wqkvT = nc.dram_tensor("wqkvT", [D, 6 * P], BF16, kind="ExternalInput")
    woT = nc.dram_tensor("woT", [CPC, D], BF16, kind="ExternalInput")
    cosT = nc.dram_tensor("cosT", [HD, S], BF16, kind="ExternalInput")
    sinT = nc.dram_tensor("sinT", [HD, S], BF16, kind="ExternalInput")
    rotL = nc.dram_tensor("rotL", [HD, HD], BF16, kind="ExternalInput")
    trimask = nc.dram_tensor("trimask", [P, P], BF16, kind="ExternalInput")
    ones = nc.dram_tensor("ones", [P, P], BF16, kind="ExternalInput")
    yT = nc.dram_tensor("yT", [B, D, S], BF16, kind="ExternalOutput")

    xTr = xT.rearrange("b (o p) s -> b p o s", p=P)
    wqr = wqkvT.rearrange("(o p) e -> p o e", p=P)

    with tile.TileContext(nc) as tc:
        with tc.tile_pool(name="pacc", bufs=2, space="PSUM") as paccp, \
             tc.tile_pool(name="ps", bufs=4, space="PSUM") as psp, \
             tc.tile_pool(name="const", bufs=1) as constp, \
             tc.tile_pool(name="xp", bufs=8) as xp, \
             tc.tile_pool(name="qk", bufs=1) as qkp, \
             tc.tile_pool(name="vp", bufs=1) as vp, \
             tc.tile_pool(name="op", bufs=1) as op_, \
             tc.tile_pool(name="attn", bufs=18) as attnp, \
             tc.tile_pool(name="tmp", bufs=2) as tmpp, \
             tc.tile_pool(name="rcp", bufs=2) as rcpp, \
             tc.tile_pool(name="yt", bufs=2) as ytp:

            # ---- constants; wqkv split per d-tile so matmuls start early ----
            wq_sb = constp.tile([P, NDT, 6 * P], BF16)
            for dt in range(NDT):
                nc.sync.dma_start(wq_sb[:, dt, :], wqr[:, dt, :])
            cos_sb = constp.tile([P, S], BF16)
            sin_sb = constp.tile([P, S], BF16)
            rot_sb = constp.tile([P, P], BF16)
            nc.scalar.dma_start(rot_sb[:], rotL[:])
            nc.scalar.dma_start(cos_sb[:], cosT[:])
            nc.scalar.dma_start(sin_sb[:], sinT[:])
            mask_sb = constp.tile([P, P], BF16)
            ones_sb = constp.tile([P, P], BF16)
            wo_sb = constp.tile([P, CPC // P, D], BF16)
            nc.scalar.dma_start(mask_sb[:], trimask[:])
            nc.scalar.dma_start(ones_sb[:], ones[:])
            nc.scalar.dma_start(wo_sb[:], woT.rearrange("(o p) e -> p o e", p=P))

            for b in range(B):
                # ---- projections (+ fused RoPE) ----
                # qkT[e] for e in {q_h0, q_h1, k_h0, k_h1}: [128, S] transposed
                qkT = [qkp.tile([P, S], BF16, tag=f"qk{e}", name=f"qkT{e}")
                       for e in range(4)]
                # v natural [s_in=128, s_out=16, ch=256]
                v_sb = vp.tile([P, NDT, CPC], BF16, tag="v")
                for half in range(NW):
                    soff = half * W
                    sl = slice(soff, soff + W)
                    xts = []
                    for g in range(NDT // XG):
                        xt = xp.tile([P, XG, W], BF16, tag="xt")
                        nc.gpsimd.dma_start(
                            xt[:], xTr[b, :, g * XG:(g + 1) * XG, sl])
                        xts.append(xt)
                    for e in range(4):
                        pa = paccp.tile([P, W], F32, tag="pacc")
                        for dt in range(NDT):
                            wt = wq_sb[:, dt, e * P:(e + 1) * P]
                            xv = xts[dt // XG][:, dt % XG, :]
                            nc.tensor.matmul(pa[:, :SC], wt, xv[:, :SC],
                                             start=(dt == 0), stop=(dt == NDT - 1),
                                             skip_group_check=True)
                            nc.tensor.matmul(pa[:, SC:], wt, xv[:, SC:],
                                             start=(dt == 0), stop=(dt == NDT - 1),
                                             skip_group_check=True)
                        nc.scalar.copy(qkT[e][:, sl], pa[:])
                        # RoPE: qrotT = RotL.T @ qT (signed pair swap), then
                        # q = qT*cos + qrotT*sin, all on this chunk
                        pr0 = psp.tile([P, SC], F32, tag="ps")
                        pr1 = psp.tile([P, SC], F32, tag="ps")
                        nc.tensor.matmul(pr0[:], rot_sb[:],
                                         qkT[e][:, soff:soff + SC],
                                         start=True, stop=True)
                        nc.tensor.matmul(pr1[:], rot_sb[:],
                                         qkT[e][:, soff + SC:soff + W],
                                         start=True, stop=True)
                        tmp = tmpp.tile([P, W], BF16, tag="ropetmp")
                        nc.vector.tensor_tensor(tmp[:, :SC], pr0[:],
                                                sin_sb[:, soff:soff + SC],
                                                AluOpType.mult)
                        nc.vector.tensor_tensor(tmp[:, SC:], pr1[:],
                                                sin_sb[:, soff + SC:soff + W],
                                                AluOpType.mult)
                        nc.vector.tensor_tensor(qkT[e][:, sl], qkT[e][:, sl],
                                                cos_sb[:, sl], AluOpType.mult)
                        nc.vector.tensor_tensor(qkT[e][:, sl], qkT[e][:, sl],
                                                tmp[:], AluOpType.add)
                    # v natural: lhsT = x s-block, rhs = wv columns
                    for ss in range(W // P):
                        pv = psp.tile([P, SC], F32, tag="ps")
                        for dt in range(NDT):
                            nc.tensor.matmul(
                                pv[:, :CPC],
                                xts[dt // XG][:, dt % XG, ss * P:(ss + 1) * P],
                                wq_sb[:, dt, 4 * P:6 * P],
                                start=(dt == 0), stop=(dt == NDT - 1))
                        nc.scalar.copy(v_sb[:, half * (W // P) + ss, :],
                                       pv[:, :CPC])

                # ---- attention: j outer, heads interleaved ----
                oT = op_.tile([P, HPC, S], BF16, tag="o")
                for j in range(NSC):
                    jsl = slice(j * SC, (j + 1) * SC)
                    n_kv = (SC // P) * (j + 1)
                    for h in range(HPC):
                        qTh, kTh = qkT[h], qkT[2 + h]
                        pa = paccp.tile([P, W], F32, tag="pacc")
                        po = pa[:, :SC]
                        prs = pa[:, SC:]
                        attns = []
                        dlts = []
                        for t in range(n_kv):
                            dp = t - (SC // P) * j
                            dlt = max(dp, 0) * P  # first valid column
                            vsl = slice(j * SC + dlt, (j + 1) * SC)
                            pscore = psp.tile([P, SC], F32, tag="ps")
                            nc.tensor.matmul(pscore[:, dlt:],
                                             kTh[:, t * P:(t + 1) * P],
                                             qTh[:, vsl],
                                             start=True, stop=True)
                            attn = attnp.tile([P, SC], BF16, tag="attn")
                            nc.scalar.activation(attn[:, dlt:], pscore[:, dlt:],
                                                 Exp, bias=0.0, scale=1.0)
                            if dp >= 0:  # triangle mask on the 128-col band
                                nc.vector.tensor_tensor(
                                    attn[:, dlt:dlt + P], attn[:, dlt:dlt + P],
                                    mask_sb[:], AluOpType.mult)
                            nc.tensor.matmul(po[:, dlt:],
                                             v_sb[:, t, h * HD:(h + 1) * HD],
                                             attn[:, dlt:],
                                             start=(t == 0), stop=(t == n_kv - 1),
                                             skip_group_check=True)
                            attns.append(attn)
                            dlts.append(dlt)
                        # deferred rowsums: `ones` stays stationary in the PE
                        for t in range(n_kv):
                            nc.tensor.matmul(prs[:, dlts[t]:], ones_sb[:],
                                             attns[t][:, dlts[t]:],
                                             start=(t == 0), stop=(t == n_kv - 1),
                                             skip_group_check=True)
                        recip = rcpp.tile([P, SC], F32, tag="recip")
                        nc.vector.reciprocal_approx_fast(recip[:], prs)
                        nc.vector.tensor_tensor(oT[:, h, jsl], po, recip[:],
                                                AluOpType.mult)

                # ---- output projection: wo tiles stationary over 4 s-chunks
                for et in range(NDT):
                    pys = [psp.tile([P, SC], F32, tag="ps", name=f"py{j}")
                           for j in range(NSC)]
                    for ct in range(HPC):
                        for j in range(NSC):
                            nc.tensor.matmul(
                                pys[j][:],
                                wo_sb[:, ct, et * P:(et + 1) * P],
                                oT[:, ct, j * SC:(j + 1) * SC],
                                start=(ct == 0), stop=(ct == HPC - 1),
                                skip_group_check=True)
                    yt = ytp.tile([P, S], BF16, tag="yt")
                    for j in range(NSC):
                        jsl = slice(j * SC, (j + 1) * SC)
                        if j % 2 == 0:
                            nc.scalar.copy(yt[:, jsl], pys[j][:])
                        else:
                            nc.vector.tensor_scalar_add(yt[:, jsl], pys[j][:],
                                                        0.0)
                    nc.sync.dma_start(yT[b, et * P:(et + 1) * P, :], yt[:])
    nc.finalize()
    return nc


def _host_inputs(x, wq, wk, wv, wo):
    """Build per-core input maps (host-side shard + transform)."""
    import ml_dtypes
    bf16 = ml_dtypes.bfloat16
    scale = 1.0 / np.sqrt(np.float32(HD))

    xTr = np.ascontiguousarray(x.transpose(0, 2, 1)).astype(bf16)

    # RoPE tables in [e, s] layout (same for every head)
    inv_freq = 1.0 / (ROPE_THETA ** (np.arange(0, HD, 2, dtype=np.float64) / HD))
    ang = np.arange(S, dtype=np.float64)[None, :] * inv_freq[:, None]  # [64, S]
    cosT = np.repeat(np.cos(ang), 2, axis=0).astype(bf16)  # [128, S]
    sinT = np.repeat(np.sin(ang), 2, axis=0).astype(bf16)

    # signed pair-swap: qrot[2i] = -q[2i+1], qrot[2i+1] = q[2i]
    # matmul computes qrot[m, s] = sum_k rotL[k, m] q[k, s]
    rotL = np.zeros((HD, HD), dtype=np.float32)
    for i in range(HD // 2):
        rotL[2 * i + 1, 2 * i] = -1.0
        rotL[2 * i, 2 * i + 1] = 1.0
    rotL = rotL.astype(bf16)

    r = np.arange(P)[:, None]
    c = np.arange(P)[None, :]
    trimask = (c >= r).astype(bf16)  # [128,128] upper-right valid

    wq_s = (wq * scale).astype(bf16)
    wk_s = wk.astype(bf16)
    wv_s = wv.astype(bf16)
    wo_s = wo.astype(bf16)

    in_maps = []
    for cix in range(NCORES):
        rows = slice(cix * CPC, (cix + 1) * CPC)  # head-channel rows
        blocks = []
        for h in range(HPC):
            hr = slice((cix * HPC + h) * HD, (cix * HPC + h + 1) * HD)
            blocks.append(wq_s[hr])   # q_h: [128, D]
        for h in range(HPC):
            hr = slice((cix * HPC + h) * HD, (cix * HPC + h + 1) * HD)
            blocks.append(wk_s[hr])
        blocks.append(wv_s[rows])     # v both heads: [256, D]
        wqkvT = np.ascontiguousarray(
            np.concatenate(blocks, axis=0).T)  # [D, 768]
        woT = np.ascontiguousarray(wo_s[:, rows].T)  # [256, D]
        in_maps.append({
            "xT": xTr,
            "wqkvT": wqkvT,
            "woT": woT,
            "cosT": cosT,
            "sinT": sinT,
            "rotL": rotL,
            "trimask": trimask,
            "ones": np.ones((P, P), dtype=bf16),
        })
    return in_maps


def _get_nc():
    global _nc_cache
    if _nc_cache is None:
        _nc_cache = _build_nc()
    return _nc_cache


def kernel(x, wq, wk, wv, wo, _trace=False):
    global last_exec_time_ns
    nc = _get_nc()
    in_maps = _host_inputs(np.asarray(x, dtype=np.float32),
                           np.asarray(wq, dtype=np.float32),
                           np.asarray(wk, dtype=np.float32),
                           np.asarray(wv, dtype=np.float32),
                           np.asarray(wo, dtype=np.float32))
    res = run_bass_kernel_spmd(nc, in_maps, core_ids=list(range(NCORES)),
                               trace=_trace)
    last_exec_time_ns = res.exec_time_ns
    y = np.zeros((B, D, S), dtype=np.float32)
    for cix in range(NCORES):
        y += res.results[cix]["yT"].astype(np.float32)
    return np.ascontiguousarray(y.transpose(0, 2, 1))


# revision 8
# speedup vs baseline: 1.2828x; 1.0281x over previous
"""Multi-head causal self-attention with RoPE on 8 Trainium2 NeuronCores.

Problem: x[2,2048,2048], wq/wk/wv/wo[2048,2048] fp32, 16 heads (hd=128),
interleaved RoPE, causal softmax, Megatron-style tensor parallelism over
heads: 2 heads per core, wo row-sharded, partial outputs summed on host.

v3: all-bf16 matmul datapath, PE-saturating schedule:
  - host converts x/weights to bf16; x resident in SBUF per batch as
    per-d-tile tiles; first chunk runs dt-outer so compute starts with
    the first arriving x tile (DMA supply-bound start)
  - projections: each weight tile streams 2x512 cols per load; q/k
    transposed [e,s] with fused RoPE (rope matmuls issued one e-tile
    late so the PSUM->SBUF copy latency is hidden); v natural
  - attention per (b, j-block, head): kv tiles processed in pairs
    sharing a [128,1024] PSUM tile so one exp instruction covers both;
    AV accumulation per tile; rowsum matmuls deferred (`ones` stays
    stationary), full pairs summed in one N=1024 matmul
  - output projection: wo stationary across 4 s-chunks, paired PSUM
    tiles alternating between the two PSUM pools; y partials in bf16
"""

import os
import sys

for _p in ("/opt/trn_rl_repo", "/root/.axon_site/_ro/trn_rl_repo"):
    if os.path.isdir(_p) and _p not in sys.path:
        sys.path.append(_p)

import numpy as np

import concourse.bacc as bacc
import concourse.mybir as mybir
import concourse.tile as tile
from concourse.alu_op_type import AluOpType
from concourse.bass_utils import run_bass_kernel_spmd

F32 = mybir.dt.float32
BF16 = mybir.dt.bfloat16

B, S, D = 2, 2048, 2048
H, HD = 16, 128
NCORES = 8
HPC = H // NCORES            # heads per core = 2
CPC = HPC * HD               # channels per core = 256
P = 128
SC = 512                     # q-block for attention / PSUM bank width
NSC = S // SC                # 4
W = 1024                     # projection s-chunk width
NW = S // W                  # 2 chunks per batch
NDT = D // P                 # 16 contraction tiles
ROPE_THETA = 10000.0

Exp = mybir.ActivationFunctionType.Exp

last_exec_time_ns = None
_nc_cache = None


def _build_nc():
    nc = bacc.Bacc("TRN2", target_bir_lowering=False, debug=False)

    xT = nc.dram_tensor("xT", [B, D, S], BF16, kind="ExternalInput")
    wqkvT = nc.dram_tensor("wqkvT", [D, 6 * P], BF16, kind="ExternalInput")
    woT = nc.dram_tensor("woT", [CPC, D], BF16, kind="ExternalInput")
    cosT = nc.dram_tensor("cosT", [HD, S], BF16, kind="ExternalInput")
    sinT = nc.dram_tensor("sinT", [HD, S], BF16, kind="ExternalInput")
    rotL = nc.dram_tensor("rotL", [HD, HD], BF16, kind="ExternalInput")
    trimask = nc.dram_tensor("trimask", [P, P], BF16, kind="ExternalInput")
    ones = nc.dram_tensor("ones", [P, P], BF16, kind="ExternalInput")
    yT = nc.dram_tensor("yT", [B, D, S], BF16, kind="ExternalOutput")

    xTr = xT.rearrange("b (o p) s -> b p o s", p=P)
    wqr = wqkvT.rearrange("(o p) e -> p o e", p=P)

    with tile.TileContext(nc) as tc:
        with tc.tile_pool(name="pacc", bufs=2, space="PSUM") as paccp, \
             tc.tile_pool(name="ps2", bufs=2, space="PSUM") as ps2p, \
             tc.tile_pool(name="const", bufs=1) as constp, \
             tc.tile_pool(name="xp", bufs=32) as xp, \
             tc.tile_pool(name="qk", bufs=1) as qkp, \
             tc.tile_pool(name="vp", bufs=1) as vp, \
             tc.tile_pool(name="op", bufs=1) as op_, \
             tc.tile_pool(name="attn", bufs=12) as attnp, \
             tc.tile_pool(name="tmp", bufs=2) as tmpp, \
             tc.tile_pool(name="rcp", bufs=2) as rcpp, \
             tc.tile_pool(name="yt", bufs=4) as ytp:

            # ---- constants; wqkv split per d-tile so matmuls start early ----
            wq_sb = constp.tile([P, NDT, 6 * P], BF16)
            for dt in range(NDT):
                nc.sync.dma_start(wq_sb[:, dt, :], wqr[:, dt, :])
            cos_sb = constp.tile([P, S], BF16)
            sin_sb = constp.tile([P, S], BF16)
            rot_sb = constp.tile([P, P], BF16)
            nc.scalar.dma_start(rot_sb[:], rotL[:])
            nc.scalar.dma_start(cos_sb[:], cosT[:])
            nc.scalar.dma_start(sin_sb[:], sinT[:])
            mask_sb = constp.tile([P, P], BF16)
            ones_sb = constp.tile([P, P], BF16)
            wo_sb = constp.tile([P, CPC // P, D], BF16)
            nc.scalar.dma_start(mask_sb[:], trimask[:])
            nc.scalar.dma_start(ones_sb[:], ones[:])
            nc.scalar.dma_start(wo_sb[:], woT.rearrange("(o p) e -> p o e", p=P))

            for b in range(B):
                # ---- projections (+ fused RoPE) ----
                # qkT[e] for e in {q_h0, q_h1, k_h0, k_h1}: [128, S] transposed
                qkT = [qkp.tile([P, S], BF16, tag=f"qk{e}", name=f"qkT{e}")
                       for e in range(4)]
                # v natural [s_in=128, s_out=16, ch=256]
                v_sb = vp.tile([P, NDT, CPC], BF16, tag="v")

                def qk_mms(pa, e, dt, xt, first, last):
                    wt = wq_sb[:, dt, e * P:(e + 1) * P]
                    nc.tensor.matmul(pa[:, :SC], wt, xt[:, :SC],
                                     start=first, stop=last,
                                     skip_group_check=True)
                    nc.tensor.matmul(pa[:, SC:], wt, xt[:, SC:],
                                     start=first, stop=last,
                                     skip_group_check=True)

                def rope(e, soff):
                    # qrotT = RotL.T @ qT (signed pair swap), then
                    # q = qT*cos + qrotT*sin, on chunk [soff:soff+W]
                    sl = slice(soff, soff + W)
                    pr = ps2p.tile([P, W], F32, tag="ps2", name="pr")
                    nc.tensor.matmul(pr[:, :SC], rot_sb[:],
                                     qkT[e][:, soff:soff + SC],
                                     start=True, stop=True)
                    nc.tensor.matmul(pr[:, SC:], rot_sb[:],
                                     qkT[e][:, soff + SC:soff + W],
                                     start=True, stop=True)
                    tmp = tmpp.tile([P, W], BF16, tag="ropetmp")
                    nc.vector.tensor_tensor(tmp[:, :SC], pr[:, :SC],
                                            sin_sb[:, soff:soff + SC],
                                            AluOpType.mult)
                    nc.vector.tensor_tensor(tmp[:, SC:], pr[:, SC:],
                                            sin_sb[:, soff + SC:soff + W],
                                            AluOpType.mult)
                    nc.vector.tensor_tensor(qkT[e][:, sl], qkT[e][:, sl],
                                            cos_sb[:, sl], AluOpType.mult)
                    nc.vector.tensor_tensor(qkT[e][:, sl], qkT[e][:, sl],
                                            tmp[:], AluOpType.add)

                for half in range(NW):
                    soff = half * W
                    sl = slice(soff, soff + W)
                    xts = []
                    for dt in range(NDT):
                        xt = xp.tile([P, W], BF16, tag="xt", name=f"xt{dt}")
                        nc.gpsimd.dma_start(xt[:], xTr[b, :, dt, sl])
                        xts.append(xt)
                    pend = []   # rope work deferred one group to hide copies
                    if b == 0 and half == 0:
                        # dt-outer over e-pairs: start compute on the first
                        # arriving x tile instead of waiting for the chunk
                        for ep in range(2):
                            pas = [paccp.tile([P, W], F32, tag="pacc",
                                              name=f"pa{i}")
                                   for i in range(2)]
                            for dt in range(NDT):
                                for i in range(2):
                                    qk_mms(pas[i], 2 * ep + i, dt, xts[dt],
                                           dt == 0, dt == NDT - 1)
                            for i in range(2):
                                nc.scalar.copy(qkT[2 * ep + i][:, sl], pas[i])
                            if ep == 1:
                                rope(0, soff)
                                rope(1, soff)
                                pend = [2, 3]
                    else:
                        for e in range(4):
                            pa = paccp.tile([P, W], F32, tag="pacc")
                            for dt in range(NDT):
                                qk_mms(pa, e, dt, xts[dt],
                                       dt == 0, dt == NDT - 1)
                            nc.scalar.copy(qkT[e][:, sl], pa[:])
                            if pend:
                                rope(pend.pop(), soff)
                            pend.append(e)
                    # v natural: lhsT = x s-block, rhs = wv columns
                    for ss in range(W // P):
                        pv = ps2p.tile([P, W], F32, tag="ps2", name="pv")
                        for dt in range(NDT):
                            nc.tensor.matmul(
                                pv[:, :CPC],
                                xts[dt][:, ss * P:(ss + 1) * P],
                                wq_sb[:, dt, 4 * P:6 * P],
                                start=(dt == 0), stop=(dt == NDT - 1))
                        nc.scalar.copy(v_sb[:, half * (W // P) + ss, :],
                                       pv[:, :CPC])
                        if pend:
                            rope(pend.pop(), soff)

                # ---- attention: j outer, heads interleaved; kv tiles in
                #      pairs sharing one [128,1024] PSUM tile -> one exp.
                #      Rowsums + normalize of each (j,h) are deferred into
                #      the next (j,h), between its first scores and first
                #      AV, so the PE has work while ACT runs the exp. ----
                oT = op_.tile([P, HPC, S], BF16, tag="o")
                pend_att = []   # [(pa, attns, h, jsl)]

                def flush_att():
                    # rowsum matmuls: `ones` stays stationary in the PE
                    pa_, attns_, h_, jsl_ = pend_att.pop()
                    prs_ = pa_[:, SC:]
                    np_ = len(attns_)
                    for tp, (attn, dla, dlb) in enumerate(attns_):
                        nc.tensor.matmul(prs_[:, dla:], ones_sb[:],
                                         attn[:, dla:SC],
                                         start=(tp == 0), stop=False,
                                         skip_group_check=True)
                        nc.tensor.matmul(prs_[:, dlb:], ones_sb[:],
                                         attn[:, SC + dlb:],
                                         start=False, stop=(tp == np_ - 1),
                                         skip_group_check=True)
                    recip = rcpp.tile([P, SC], F32, tag="recip")
                    nc.vector.reciprocal_approx_fast(recip[:], prs_)
                    nc.vector.tensor_tensor(oT[:, h_, jsl_], pa_[:, :SC],
                                            recip[:], AluOpType.mult)

                for j in range(NSC):
                    jsl = slice(j * SC, (j + 1) * SC)
                    n_kv = (SC // P) * (j + 1)
                    for h in range(HPC):
                        qTh, kTh = qkT[h], qkT[2 + h]
                        pa = paccp.tile([P, W], F32, tag="pacc")
                        po = pa[:, :SC]
                        attns = []   # (attn_pair_tile, dlt_a, dlt_b)
                        for tp in range(n_kv // 2):
                            psc = ps2p.tile([P, W], F32, tag="ps2",
                                            name="pscore")
                            attn = attnp.tile([P, W], BF16, tag="attn")
                            dls = []
                            for i in range(2):
                                t = 2 * tp + i
                                dp = t - (SC // P) * j
                                dlt = max(dp, 0) * P  # first valid column
                                dls.append(dlt)
                                vsl = slice(j * SC + dlt, (j + 1) * SC)
                                nc.tensor.matmul(
                                    psc[:, i * SC + dlt:(i + 1) * SC],
                                    kTh[:, t * P:(t + 1) * P],
                                    qTh[:, vsl],
                                    start=True, stop=True)
                            # one exp for both tiles; cleared PSUM regions
                            # produce exp(0)=1 garbage that is never read
                            nc.scalar.activation(attn[:], psc[:],
                                                 Exp, bias=0.0, scale=1.0)
                            for i in range(2):
                                t = 2 * tp + i
                                dp = t - (SC // P) * j
                                dlt = dls[i]
                                if dp >= 0:  # triangle mask on diag band
                                    bnd = slice(i * SC + dlt,
                                                i * SC + dlt + P)
                                    nc.vector.tensor_tensor(
                                        attn[:, bnd], attn[:, bnd],
                                        mask_sb[:], AluOpType.mult)
                            if tp == 0 and pend_att:
                                flush_att()
                            for i in range(2):
                                t = 2 * tp + i
                                dlt = dls[i]
                                nc.tensor.matmul(
                                    po[:, dlt:],
                                    v_sb[:, t, h * HD:(h + 1) * HD],
                                    attn[:, i * SC + dlt:(i + 1) * SC],
                                    start=(t == 0), stop=(t == n_kv - 1),
                                    skip_group_check=True)
                            attns.append((attn, dls[0], dls[1]))
                        pend_att.append((pa, attns, h, jsl))
                if pend_att:
                    flush_att()

                # ---- output projection: wo tiles stationary over 4
                #      s-chunks; paired PSUM tiles alternate pools ----
                for et in range(NDT):
                    pool = paccp if et % 2 == 0 else ps2p
                    tg = "pacc" if et % 2 == 0 else "ps2"
                    pys = [pool.tile([P, W], F32, tag=tg, name=f"py{i}")
                           for i in range(2)]
                    for ct in range(HPC):
                        for j in range(NSC):
                            nc.tensor.matmul(
                                pys[j // 2][:, (j % 2) * SC:(j % 2 + 1) * SC],
                                wo_sb[:, ct, et * P:(et + 1) * P],
                                oT[:, ct, j * SC:(j + 1) * SC],
                                start=(ct == 0), stop=(ct == HPC - 1),
                                skip_group_check=True)
                    yt = ytp.tile([P, S], BF16, tag="yt")
                    nc.scalar.copy(yt[:, :W], pys[0][:])
                    nc.vector.tensor_scalar_add(yt[:, W:], pys[1][:], 0.0)
                    nc.sync.dma_start(yT[b, et * P:(et + 1) * P, :], yt[:])
    nc.finalize()
    return nc


def _host_inputs(x, wq, wk, wv, wo):
    """Build per-core input maps (host-side shard + transform)."""
    import ml_dtypes
    bf16 = ml_dtypes.bfloat16
    scale = 1.0 / np.sqrt(np.float32(HD))

    xTr = np.ascontiguousarray(x.transpose(0, 2, 1)).astype(bf16)

    # RoPE tables in [e, s] layout (same for every head)
    inv_freq = 1.0 / (ROPE_THETA ** (np.arange(0, HD, 2, dtype=np.float64) / HD))
    ang = np.arange(S, dtype=np.float64)[None, :] * inv_freq[:, None]  # [64, S]
    cosT = np.repeat(np.cos(ang), 2, axis=0).astype(bf16)  # [128, S]
    sinT = np.repeat(np.sin(ang), 2, axis=0).astype(bf16)

    # signed pair-swap: qrot[2i] = -q[2i+1], qrot[2i+1] = q[2i]
    # matmul computes qrot[m, s] = sum_k rotL[k, m] q[k, s]
    rotL = np.zeros((HD, HD), dtype=np.float32)
    for i in range(HD // 2):
        rotL[2 * i + 1, 2 * i] = -1.0
        rotL[2 * i, 2 * i + 1] = 1.0
    rotL = rotL.astype(bf16)

    r = np.arange(P)[:, None]
    c = np.arange(P)[None, :]
    trimask = (c >= r).astype(bf16)  # [128,128] upper-right valid

    wq_s = (wq * scale).astype(bf16)
    wk_s = wk.astype(bf16)
    wv_s = wv.astype(bf16)
    wo_s = wo.astype(bf16)

    in_maps = []
    for cix in range(NCORES):
        rows = slice(cix * CPC, (cix + 1) * CPC)  # head-channel rows
        blocks = []
        for h in range(HPC):
            hr = slice((cix * HPC + h) * HD, (cix * HPC + h + 1) * HD)
            blocks.append(wq_s[hr])   # q_h: [128, D]
        for h in range(HPC):
            hr = slice((cix * HPC + h) * HD, (cix * HPC + h + 1) * HD)
            blocks.append(wk_s[hr])
        blocks.append(wv_s[rows])     # v both heads: [256, D]
        wqkvT = np.ascontiguousarray(
            np.concatenate(blocks, axis=0).T)  # [D, 768]
        woT = np.ascontiguousarray(wo_s[:, rows].T)  # [256, D]
        in_maps.append({
            "xT": xTr,
            "wqkvT": wqkvT,
            "woT": woT,
            "cosT": cosT,
            "sinT": sinT,
            "rotL": rotL,
            "trimask": trimask,
            "ones": np.ones((P, P), dtype=bf16),
        })
    return in_maps


def _get_nc():
    global _nc_cache
    if _nc_cache is None:
        _nc_cache = _build_nc()
    return _nc_cache


def kernel(x, wq, wk, wv, wo, _trace=False):
    global last_exec_time_ns
    nc = _get_nc()
    in_maps = _host_inputs(np.asarray(x, dtype=np.float32),
                           np.asarray(wq, dtype=np.float32),
                           np.asarray(wk, dtype=np.float32),
                           np.asarray(wv, dtype=np.float32),
                           np.asarray(wo, dtype=np.float32))
    res = run_bass_kernel_spmd(nc, in_maps, core_ids=list(range(NCORES)),
                               trace=_trace)
    last_exec_time_ns = res.exec_time_ns
    y = np.zeros((B, D, S), dtype=np.float32)
    for cix in range(NCORES):
        y += res.results[cix]["yT"].astype(np.float32)
    return np.ascontiguousarray(y.transpose(0, 2, 1))
